# revision 32
# baseline (speedup 1.0000x reference)
"""Trainium2 Bass kernel for causal multi-head attention with RoPE.

nn_CausalAttention: x [2, 2048, 2048], Wq/Wk/Wv [2048, 2048] (y = x @ W.T),
16 heads of dim 128, RoPE, causal fp32 softmax.

Sharding (tensor-parallel heads): each of the 8 NeuronCores owns 2 heads (a
256-wide slice of the QKV output dim) for both batch elements. The full
output is assembled on host by concatenating per-core feature slices.

Fully software-pipelined single-stream schedule (measured ~257us vs the
337us fp32r baseline; PE ~100% busy t=20..240us of a ~257us run):
  * All matmul operands are fp16 with fp32 PSUM accumulation (rel err
    ~6e-4 vs the fp32 reference; fp8 was tested in simulation and fails
    the 2e-2 gate at ~5e-2 because q/k quantization noise shifts peaked
    softmax rows). fp16 streams at the same 1 col/cycle PE rate as
    fp32r/bf16 but halves DMA/SBUF and unlocks 16-bit DVE fast modes.
  * One stream of projection slabs (Q proj -> RoPE -> V s-tiles 0,1 ->
    K proj -> RoPE -> V s-tiles 2,3 per 512-position slab), with V
    projected directly into [seq, dim] layout (x-chunk stationary, Wv^T
    moving) so no PE transposes are needed.
  * Causal-attention "tile groups" (score-matmul -> exp -> tri-mask ->
    DVE denominator accumulate -> attn@V, 4-deep pend queue) are fed
    between projection matmuls by a Feeder queue as soon as their slab
    dependencies are emitted. This spreads the scalar-engine exp work
    across the whole timeline so the PE almost never waits on exp, and
    the last batch's attention has no exposed exp-bound window.
  * The softmax denominator is accumulated on the DVE in fp16 (16-bit
    all-SBUF ops run 2-4x) and reduced across the 128 key lanes with a
    single ones-matmul per 512-query tile (vs one per key tile),
    removing ~30us of PE streaming.
  * exp computes e^(s*scale - 4): the bias cancels in the softmax ratio
    and keeps fp16 exp values far from overflow.
  * Diagonal score tiles sub-range to exact 128-col multiples; the
    normalize+store runs in 256-col halves so the output DMA overlaps
    the second half.

PSUM budget (8 banks): Q/K accumulators 2 (bufs=1, evicted to fp16 by DVE
before reuse), V pair-accumulator 1 (two 256-col s-tiles share one bank and
one accumulation group), scores 2, attention output 2, denominator 1.
"""

import math
from collections import deque

import numpy as np

import concourse.bacc as bacc
import concourse.bass as bass
import concourse.mybir as mybir
import concourse.tile as tile
from concourse import bass_utils

F32 = mybir.dt.float32
F16 = mybir.dt.float16
AF = mybir.ActivationFunctionType

S = 2048
M = 2048
NCORES = 8

D = 128          # head dim
NH = 2           # heads per core
NB = 2           # batches
SLAB = 512       # phase-A sequence slab == phase-B query tile
QT = 512
NE = S // SLAB
EXP_BIAS = -4.0  # exp(s*scale + EXP_BIAS); cancels in softmax ratio


def _rope_perm(n):
    """Row permutation for the quadrant-16 RoPE layout.

    New row p (within a 128-row head block): quadrant qd = p//32, r = p%32.
    r < 16  -> even element of pair i = 16*qd + r      (old row 2i)
    r >= 16 -> odd  element of pair i = 16*qd + (r-16) (old row 2i+1)
    Pair elements are 16 partitions apart inside one 32-partition quadrant,
    so the RoPE combine is a stream_shuffle with a 16-rotation mask.
    """
    perm = []
    for hb in range(n // D):
        base = hb * D
        for qd in range(4):
            perm += [base + 2 * (16 * qd + r) for r in range(16)]
            perm += [base + 2 * (16 * qd + r) + 1 for r in range(16)]
    return np.array(perm)


SWAP16 = [(i + 16) % 32 for i in range(32)]

_HOST_CACHE = {}


def _host_shared(x):
    """fp16 conversions shared by all 8 cores (computed once per input set).
    The cache holds a reference to x so its id() stays unique while cached."""
    key = id(x)
    if key in _HOST_CACHE:
        return _HOST_CACHE[key][0]
    theta = np.exp(
        -np.float32(np.log(10000.0))
        * (np.arange(0, D, 2, dtype=np.float32) / np.float32(D))
    ).astype(np.float32)
    pos = np.arange(S, dtype=np.float32)
    freqs = theta[:, None] * pos[None, :]  # [64, S]
    cos_t, sin_t = np.cos(freqs), np.sin(freqs)
    p = np.arange(128)
    i_of_p = 16 * (p // 32) + (p % 16)
    is_odd = (p % 32) >= 16
    packC = cos_t[i_of_p].astype(np.float16)                     # [128, S]
    packS = np.where(
        is_odd[:, None], -sin_t[i_of_p], sin_t[i_of_p]
    ).astype(np.float16)

    kk, qq = np.meshgrid(np.arange(128), np.arange(128), indexing="ij")
    tri = (kk <= qq).astype(np.float16)

    shared = {
        "xT0": np.ascontiguousarray(x[0].T).astype(np.float16),
        "xT1": np.ascontiguousarray(x[1].T).astype(np.float16),
        "packC": packC,
        "packS": packS,
        "tri": tri,
        "ones": np.ones((128, 128), dtype=np.float16),
        "bias4": np.full((128, 1), EXP_BIAS, dtype=np.float32),
    }
    _HOST_CACHE.clear()
    _HOST_CACHE[key] = (shared, x)
    return shared


def prep_core_inputs(x, Wq, Wk, Wv, core, S, M):
    """Host-side shard prep for one core. x [2,S,M], W* [M', M] where
    rows [core*256, core*256+256) of W* are this core's heads."""
    shared = _host_shared(x)
    nsl = slice(core * NH * D, (core + 1) * NH * D)
    perm = _rope_perm(NH * D)
    io = dict(shared)
    io["wqT"] = np.ascontiguousarray(Wq[nsl][perm].T).astype(np.float16)
    io["wkT"] = np.ascontiguousarray(Wk[nsl][perm].T).astype(np.float16)
    io["wvT"] = np.ascontiguousarray(Wv[nsl].T).astype(np.float16)
    return io


class Feeder:
    """Queue of deferred attention tile-group emitters, drained between
    projection matmul units so exp latency hides under PE work."""

    def __init__(self, units_per_slab):
        self.q = deque()
        self.units_per_slab = units_per_slab
        self.cnt = 0
        self.k = units_per_slab

    def enqueue(self, fns):
        self.q.extend(fns)
        self.k = max(1, self.units_per_slab // max(1, len(self.q)))

    def tick(self):
        self.cnt += 1
        if self.q and self.cnt % self.k == 0:
            self.q.popleft()()

    def drain(self):
        while self.q:
            self.q.popleft()()


def build_attention(tc: tile.TileContext, io: dict, S: int, M: int):
    nc = tc.nc
    MC = M // 128
    scale = 1.0 / math.sqrt(D)
    xT = [io["xT0"], io["xT1"]]
    outT = io["outT"]

    with (
        tc.tile_pool(name="wpool", bufs=1) as wpool,
        tc.tile_pool(name="constpool", bufs=1) as constpool,
        tc.tile_pool(name="xp", bufs=2) as xpool,
        tc.tile_pool(name="rope", bufs=2) as ropetmp,
        tc.tile_pool(name="pack", bufs=1) as packpool,
        tc.tile_pool(name="qkv", bufs=2) as qkvp,
        tc.tile_pool(name="expp", bufs=8) as expp,
        tc.tile_pool(name="denp", bufs=2) as denp,
        tc.tile_pool(name="outp", bufs=2) as outp,
        tc.tile_pool(name="psqk", bufs=1, space="PSUM") as psqk,
        tc.tile_pool(name="psv", bufs=1, space="PSUM") as psvp,
        tc.tile_pool(name="psS", bufs=2, space="PSUM") as psS,
        tc.tile_pool(name="psO", bufs=2, space="PSUM") as psO,
        tc.tile_pool(name="psD", bufs=1, space="PSUM") as psDen,
    ):
        w_sb = {}
        for name in ("wqT", "wkT", "wvT"):
            w_sb[name] = wpool.tile([128, MC, NH * D], F16, tag=name, name=name)
        tri_sb = constpool.tile([128, 128], F16)
        ones_sb = constpool.tile([128, 128], F16)
        bias_sb = constpool.tile([128, 1], F32)

        # projection PE ticks per slab: 16 Q + 16 V01 + 16 K + 16 V23
        feeder = Feeder(units_per_slab=64)

        # ---------------- attention tile-group machinery ----------------
        def make_qt_state(b, qT_sb, kT_sb, v_sb):
            """Per-(b,qt) state: for each head an out accumulator, den
            accumulator and av pend queue, created lazily at kt=0."""
            return {"b": b, "qT": qT_sb, "kT": kT_sb, "v": v_sb,
                    "out_ps": {}, "den": {}, "pend": {h: [] for h in range(NH)}}

        def make_group(st, h, qt, kt, nkt):
            def emit():
                qT_sb, kT_sb, v_sb = st["qT"], st["kT"], st["v"]
                j = kt - (nkt - 4)
                rs = 128 * j if j > 0 else 0
                if kt == 0:
                    st["out_ps"][h] = psO.tile([128, QT], F32, tag="out",
                                               name="out_ps")
                    st["den"][h] = denp.tile([128, QT], F16, tag=f"den{h}",
                                             name="den_acc")
                out_ps, den_acc = st["out_ps"][h], st["den"][h]
                pend = st["pend"][h]

                s_ps = psS.tile([128, QT], F32, tag="s", name="s_ps")
                nc.tensor.matmul(
                    s_ps[:, rs:],
                    kT_sb[:, h, kt * 128:(kt + 1) * 128],
                    qT_sb[:, h, qt * QT + rs:(qt + 1) * QT],
                    start=True,
                    stop=True,
                )
                expS = expp.tile([128, QT], F16, tag=f"exp{h}", name="expS")
                nc.scalar.activation(
                    expS[:, rs:], s_ps[:, rs:], AF.Exp,
                    bias=bias_sb[:], scale=scale,
                )
                if j >= 0:
                    nc.vector.tensor_mul(
                        expS[:, 128 * j:128 * (j + 1)],
                        expS[:, 128 * j:128 * (j + 1)],
                        tri_sb[:],
                    )
                if kt == 0:
                    nc.vector.tensor_copy(den_acc[:], expS[:])
                else:
                    nc.vector.tensor_add(
                        den_acc[:, rs:], den_acc[:, rs:], expS[:, rs:]
                    )
                pend.append((expS, kt, rs))
                if len(pend) > 4:
                    av_tail(st, h, nkt, *pend.pop(0))
                if kt == nkt - 1:
                    while pend:
                        av_tail(st, h, nkt, *pend.pop(0))
                    finalize(st, h, qt)
            return emit

        def av_tail(st, h, nkt, expS, kt, rs):
            nc.tensor.matmul(
                st["out_ps"][h][:, rs:],
                st["v"][:, kt, h * D:(h + 1) * D],
                expS[:, rs:],
                start=(kt == 0),
                stop=(kt == nkt - 1),
            )

        def finalize(st, h, qt):
            u = st["b"] * NH + h
            den_ps = psDen.tile([128, QT], F32, tag="denp", name="den_ps")
            nc.tensor.matmul(
                den_ps[:], ones_sb[:], st["den"][h][:], start=True, stop=True
            )
            recip = outp.tile([128, QT], F32, tag="recip", name="recip")
            nc.vector.reciprocal_approx_fast(recip[:], den_ps[:])
            o_sb = outp.tile([128, QT], F32, tag="o", name="o_sb")
            # normalize+store in halves: the first half's DMA overlaps the
            # second half's multiply, shortening the end-of-kernel chain
            for c0 in (0, QT // 2):
                c1 = c0 + QT // 2
                nc.vector.tensor_mul(
                    o_sb[:, c0:c1], st["out_ps"][h][:, c0:c1], recip[:, c0:c1]
                )
                nc.sync.dma_start(
                    outT[u, :, qt * QT + c0:qt * QT + c1], o_sb[:, c0:c1]
                )

        # ---------------- projection slab emission ----------------
        for b in range(NB):
            qT_sb = qkvp.tile([128, NH, S], F16, tag="qT", name="qT_sb")
            kT_sb = qkvp.tile([128, NH, S], F16, tag="kT", name="kT_sb")
            v_sb = qkvp.tile([128, S // 128, NH * D], F16, tag="v",
                             name="v_sb")
            xT_r = xT[b].rearrange("(mo p) s -> p mo s", p=128)

            for e in range(NE):
                sl = slice(e * SLAB, (e + 1) * SLAB)
                xe = xpool.tile([128, MC, SLAB], F16, tag="xe", name="xe")
                if b == 0 and e == 0:
                    # first slab: chunk x and the weights so the m=0 matmuls
                    # start immediately, and order streams by first use
                    # (Q chain, then V01, then K chain)
                    wr = {
                        n: io[n].rearrange("(mo p) n -> p mo n", p=128)
                        for n in ("wqT", "wkT", "wvT")
                    }
                    for m in range(4):
                        nc.sync.dma_start(
                            xe[:, m, :], xT[b][m * 128:(m + 1) * 128, sl]
                        )
                        nc.sync.dma_start(
                            w_sb["wqT"][:, m, :],
                            io["wqT"][m * 128:(m + 1) * 128, :],
                        )
                    for g in range(4, MC, 4):
                        nc.sync.dma_start(
                            xe[:, g:g + 4, :], xT_r[:, g:g + 4, sl]
                        )
                        nc.sync.dma_start(
                            w_sb["wqT"][:, g:g + 4, :], wr["wqT"][:, g:g + 4, :]
                        )
                    for g in range(0, MC, 4):
                        nc.sync.dma_start(
                            w_sb["wvT"][:, g:g + 4, :], wr["wvT"][:, g:g + 4, :]
                        )
                    for g in range(0, MC, 4):
                        nc.sync.dma_start(
                            w_sb["wkT"][:, g:g + 4, :], wr["wkT"][:, g:g + 4, :]
                        )
                    nc.sync.dma_start(tri_sb[:], io["tri"][:])
                    nc.sync.dma_start(ones_sb[:], io["ones"][:])
                    nc.sync.dma_start(bias_sb[:], io["bias4"][:])
                else:
                    nc.sync.dma_start(xe[:], xT_r[:, :, sl])
                packC = packpool.tile([128, SLAB], F16, tag="packC",
                                      name="packC")
                packS = packpool.tile([128, SLAB], F16, tag="packS",
                                      name="packS")
                nc.sync.dma_start(packC[:], io["packC"][:, sl])
                nc.sync.dma_start(packS[:], io["packS"][:, sl])

                st = make_qt_state(b, qT_sb, kT_sb, v_sb)
                nkt = (e + 1) * (QT // 128)

                def rope(ps, dst, h):
                    p16 = ropetmp.tile([128, SLAB], F16, tag="p16",
                                       name="p16")
                    t1 = ropetmp.tile([128, SLAB], F16, tag="t1", name="t1")
                    t2 = ropetmp.tile([128, SLAB], F16, tag="t2", name="t2")
                    t2s = ropetmp.tile([128, SLAB], F16, tag="t2s",
                                       name="t2s")
                    nc.vector.tensor_copy(p16[:], ps[:])
                    nc.vector.tensor_mul(t1[:], p16[:], packC[:])
                    nc.vector.tensor_mul(t2[:], p16[:], packS[:])
                    nc.vector.stream_shuffle(t2s[:], t2[:], SWAP16)
                    nc.vector.tensor_add(dst[:, h, sl], t1[:], t2s[:])

                # --- Q projection (both heads), RoPE ---
                ps = {h: psqk.tile([128, SLAB], F32, tag=f"pqk{h}",
                                   name=f"pqk{h}") for h in range(NH)}
                for m in range(MC):
                    for h in range(NH):
                        nc.tensor.matmul(
                            ps[h][:],
                            w_sb["wqT"][:, m, h * D:(h + 1) * D],
                            xe[:, m, :],
                            start=(m == 0),
                            stop=(m == MC - 1),
                        )
                    feeder.tick()
                for h in range(NH):
                    rope(ps[h], qT_sb, h)

                # Q(slab e) ready -> off-diagonal groups for qt=e (need
                # keys/values only from earlier slabs)
                groups = []
                for kt in range(4 * e):
                    for h in range(NH):
                        groups.append(make_group(st, h, e, kt, nkt))
                feeder.enqueue(groups)

                # --- V projection for s-tiles 0,1 (paired in one bank) ---
                def vpair(st0):
                    pv = psvp.tile([128, 2, NH * D], F32, tag="pv",
                                   name="pv")
                    for m in range(MC):
                        for i in range(2):
                            nc.tensor.matmul(
                                pv[:, i, :],
                                xe[:, m,
                                   (st0 + i) * 128:(st0 + i + 1) * 128],
                                w_sb["wvT"][:, m, :],
                                start=(m == 0 and i == 0),
                                stop=(m == MC - 1 and i == 1),
                                skip_group_check=True,
                            )
                        feeder.tick()
                    gst = e * (SLAB // 128) + st0
                    nc.vector.tensor_copy(v_sb[:, gst:gst + 2, :], pv[:])

                vpair(0)

                # --- K projection (both heads), RoPE ---
                for m in range(MC):
                    for h in range(NH):
                        nc.tensor.matmul(
                            ps[h][:],
                            w_sb["wkT"][:, m, h * D:(h + 1) * D],
                            xe[:, m, :],
                            start=(m == 0),
                            stop=(m == MC - 1),
                        )
                    feeder.tick()
                for h in range(NH):
                    rope(ps[h], kT_sb, h)

                # first two diagonal key tiles use V s-tiles 0,1 (already
                # evicted) -> they can interleave with the V23 chain
                groups = []
                for kt in (4 * e, 4 * e + 1):
                    for h in range(NH):
                        groups.append(make_group(st, h, e, kt, nkt))
                feeder.enqueue(groups)

                vpair(2)

                # last diagonal tiles need V s-tiles 2,3. On the very last
                # slab emit h-major so head 0's finalize chain overlaps
                # head 1's groups instead of serializing at the end.
                groups = []
                if b == NB - 1 and e == NE - 1:
                    for h in range(NH):
                        for kt in range(4 * e + 2, nkt):
                            groups.append(make_group(st, h, e, kt, nkt))
                else:
                    for kt in range(4 * e + 2, nkt):
                        for h in range(NH):
                            groups.append(make_group(st, h, e, kt, nkt))
                feeder.enqueue(groups)

        feeder.drain()


_NC_CACHE = {}


def _get_nc():
    if "nc" not in _NC_CACHE:
        nc = bacc.Bacc(
            "TRN2", target_bir_lowering=False, debug=False, num_devices=NCORES
        )
        io = {}
        for name, shape, dt_ in (
            ("xT0", [M, S], F16),
            ("xT1", [M, S], F16),
            ("wqT", [M, NH * D], F16),
            ("wkT", [M, NH * D], F16),
            ("wvT", [M, NH * D], F16),
            ("packC", [128, S], F16),
            ("packS", [128, S], F16),
            ("tri", [128, 128], F16),
            ("ones", [128, 128], F16),
            ("bias4", [128, 1], F32),
        ):
            io[name] = nc.dram_tensor(name, shape, dt_, kind="ExternalInput").ap()
        io["outT"] = nc.dram_tensor(
            "outT", [NB * NH, 128, S], F32, kind="ExternalOutput"
        ).ap()
        with tile.TileContext(nc) as tc:
            build_attention(tc, io, S, M)
        nc.compile()
        _NC_CACHE["nc"] = nc
    return _NC_CACHE["nc"]


def kernel(x, Wq, Wk, Wv):
    x = np.asarray(x, dtype=np.float32)
    Wq = np.asarray(Wq, dtype=np.float32)
    Wk = np.asarray(Wk, dtype=np.float32)
    Wv = np.asarray(Wv, dtype=np.float32)

    nc = _get_nc()
    in_maps = [prep_core_inputs(x, Wq, Wk, Wv, c, S, M) for c in range(NCORES)]
    res = bass_utils.run_bass_kernel_spmd(nc, in_maps, core_ids=list(range(NCORES)))

    out = np.empty((NB, S, M), dtype=np.float32)
    for c in range(NCORES):
        outT = res.results[c]["outT"]
        for u in range(NB * NH):
            b, hl = u // NH, u % NH
            col = c * NH * D + hl * D
            out[b, :, col:col + D] = outT[u].T
    return out
